# revision 7
# baseline (speedup 1.0000x reference)
"""Trainium2 Bass kernel for nn_AttentionBlock (B=8, T=2048, C=512).

Data-parallel over batch: one batch element per NeuronCore (8 cores).

Per-core algorithm (batch b, x: [T, C]):
  qT = (Wq @ x^T + bq)      stored [D, T]   (feature-major)
  kT = (Wk @ x^T + bk)      stored [D, T]
  v  = (x @ Wv^T + bv)      stored [T, D]   (natural)
  stT[k, q] = sum_d kT[d,k] qT[d,q]         (scores transposed: [Tk, Tq])
  masked: valid iff q >= k  (causal); softmax over q = FREE axis of stT
  e[k, q] = exp((stT - max_q) / sqrt(D)); S[k] = sum_q e
  v_scaled[k, :] = v[k, :] / S[k]
  out[q, :] = sum_k e[k, q] * v_scaled[k, :]
  output = concat([x, out], axis=-1)        [T, 2C]

Matmul inputs are stored as float32r (fp32 data, reduced-precision PE pass
at full rate); producers write the rounded dtype directly so the BIR
verifier's fp32r rounding rule is satisfied. Causal structure is
exploited: score slices entirely in the masked region are never computed,
and the attn@v accumulation is triangular.
"""

import numpy as np

import concourse.bass as bass
import concourse.mybir as mybir
import concourse.tile as tile
from concourse import bacc

B, T, C = 8, 2048, 512
D = 512                      # KEY_SIZE == VALUE_SIZE == 512
P = 128                      # partitions
NT = T // P                  # 16 t-chunks
NC4 = C // P                 # 4 contraction chunks
ND = D // P                  # 4 d-chunks
QS = 512                     # q-slice width for score matmuls
NQ = T // QS                 # 4 q-slices
SCALE = float(1.0 / np.sqrt(D))
NEG = -1.0e30

F32 = mybir.dt.float32

MM_DTYPES = {
    "f32r": mybir.dt.float32r,
    "f32": mybir.dt.float32,
    "bf16": mybir.dt.bfloat16,
}


def build_nc(mm_dtype="f32r"):
    """Build the single-core Bass program. mm_dtype in {"f32r", "f32", "bf16"}."""
    mdt = MM_DTYPES[mm_dtype]

    nc = bacc.Bacc(trn_type="TRN2", target_bir_lowering=False)

    x = nc.dram_tensor("x", [T, C], F32, kind="ExternalInput").ap()
    Wq = nc.dram_tensor("Wq", [D, C], F32, kind="ExternalInput").ap()
    bq = nc.dram_tensor("bq", [D], F32, kind="ExternalInput").ap()
    Wk = nc.dram_tensor("Wk", [D, C], F32, kind="ExternalInput").ap()
    bk = nc.dram_tensor("bk", [D], F32, kind="ExternalInput").ap()
    Wv = nc.dram_tensor("Wv", [D, C], F32, kind="ExternalInput").ap()
    bv = nc.dram_tensor("bv", [D], F32, kind="ExternalInput").ap()
    out = nc.dram_tensor("out", [T, 2 * C], F32, kind="ExternalOutput").ap()

    with tile.TileContext(nc) as tc:
        _emit(nc, tc, x, (Wq, bq), (Wk, bk), (Wv, bv), out, mdt)
    nc.compile()
    return nc


def _emit(nc, tc, x, wq, wk, wv, out, mdt):
    from contextlib import ExitStack

    Wq, bq = wq
    Wk, bk = wk
    Wv, bv = wv

    with ExitStack() as ctx:
        const = ctx.enter_context(tc.tile_pool(name="const", bufs=1))
        persist = ctx.enter_context(tc.tile_pool(name="persist", bufs=1))
        stats = ctx.enter_context(tc.tile_pool(name="stats", bufs=4))
        outsb = ctx.enter_context(tc.tile_pool(name="outsb", bufs=3))
        psum_acc = ctx.enter_context(
            tc.tile_pool(name="psum_acc", bufs=4, space="PSUM")
        )
        psum_st = ctx.enter_context(
            tc.tile_pool(name="psum_st", bufs=4, space="PSUM")
        )

        # ---- constants ----
        ident = const.tile([P, P], F32, name="ident")
        nc.gpsimd.memset(ident, 0.0)
        nc.gpsimd.affine_select(
            out=ident, in_=ident, compare_op=mybir.AluOpType.not_equal,
            fill=1.0, base=0, pattern=[[-1, P]], channel_multiplier=1,
        )
        # tri[p, j] = 0 where j >= p (valid), NEG where j < p (masked)
        tri = const.tile([P, P], F32, name="tri")
        nc.gpsimd.memset(tri, 0.0)
        nc.gpsimd.affine_select(
            out=tri, in_=tri, compare_op=mybir.AluOpType.is_ge,
            fill=NEG, base=0, pattern=[[1, P]], channel_multiplier=-1,
        )
        # ones row / bias row for the V-bias rank-1 matmul, in matmul dtype
        ones_f = const.tile([1, P], F32, name="ones_f")
        nc.gpsimd.memset(ones_f, 1.0)
        ones1 = const.tile([1, P], mdt, name="ones1")
        nc.vector.tensor_copy(ones1, ones_f)
        bv_f = const.tile([1, D], F32, name="bv_f")
        nc.sync.dma_start(out=bv_f, in_=bv.unsqueeze(0))
        bv_sb = const.tile([1, D], mdt, name="bv_sb")
        nc.vector.tensor_copy(bv_sb, bv_f)

        bq_sb = const.tile([P, ND], F32, name="bq_sb")
        bk_sb = const.tile([P, ND], F32, name="bk_sb")
        for dc in range(ND):
            nc.sync.dma_start(
                out=bq_sb[:, dc : dc + 1],
                in_=bq[dc * P : (dc + 1) * P].unsqueeze(-1),
            )
            nc.sync.dma_start(
                out=bk_sb[:, dc : dc + 1],
                in_=bk[dc * P : (dc + 1) * P].unsqueeze(-1),
            )

        # ---- x passthrough: out[:, 0:C] = x (DRAM->DRAM) ----
        for g in range(4):
            r0 = g * (T // 4)
            nc.sync.dma_start(
                out=out[r0 : r0 + T // 4, 0:C], in_=x[r0 : r0 + T // 4, :]
            )

        # ---- persistent activations (matmul dtype) ----
        qT = [persist.tile([P, T], mdt, name=f"qT{i}", tag=f"qT{i}") for i in range(ND)]
        kT = [persist.tile([P, T], mdt, name=f"kT{i}", tag=f"kT{i}") for i in range(ND)]
        v = [persist.tile([P, D], mdt, name=f"v{i}", tag=f"v{i}") for i in range(NT)]

        # ---- phase 0: load + transpose weights and x (plain fp32 PE transpose) ----
        with tc.tile_pool(name="wx", bufs=1) as wx, \
             tc.tile_pool(name="loads", bufs=1) as loads:
            wqT = [wx.tile([P, D], mdt, name=f"wqT{i}", tag=f"wqT{i}") for i in range(NC4)]
            wkT = [wx.tile([P, D], mdt, name=f"wkT{i}", tag=f"wkT{i}") for i in range(NC4)]
            wvT = [wx.tile([P, D], mdt, name=f"wvT{i}", tag=f"wvT{i}") for i in range(NC4)]
            xT = [wx.tile([P, T], mdt, name=f"xT{i}", tag=f"xT{i}") for i in range(NC4)]

            for W, wT in ((Wq, wqT), (Wk, wkT), (Wv, wvT)):
                wnat = []
                for dc in range(ND):
                    wn = loads.tile([P, C], F32, name=f"wnat{dc}", tag=f"wn{dc}")
                    nc.sync.dma_start(out=wn, in_=W[dc * P : (dc + 1) * P, :])
                    wnat.append(wn)
                for cc in range(NC4):
                    ps = psum_acc.tile([P, D], F32, name="ps_wt", tag="acc")
                    for dc in range(ND):
                        nc.tensor.transpose(
                            ps[:, dc * P : (dc + 1) * P],
                            wnat[dc][:, cc * P : (cc + 1) * P],
                            ident,
                        )
                    nc.vector.tensor_copy(wT[cc], ps)

            for tg in range(4):
                xnat = []
                for j in range(4):
                    tch = tg * 4 + j
                    xn = loads.tile([P, C], F32, name=f"xnat{j}", tag=f"xn{j}", bufs=2)
                    nc.sync.dma_start(out=xn, in_=x[tch * P : (tch + 1) * P, :])
                    xnat.append(xn)
                for cc in range(NC4):
                    ps = psum_acc.tile([P, D], F32, name="ps_xt", tag="acc")
                    for j in range(4):
                        nc.tensor.transpose(
                            ps[:, j * P : (j + 1) * P],
                            xnat[j][:, cc * P : (cc + 1) * P],
                            ident,
                        )
                    nc.vector.tensor_copy(xT[cc][:, tg * C : (tg + 1) * C], ps)

            # ---- phase 1: projections ----
            # qT[dc][:, qs] = sum_cc wqT[cc][:,dc-block].T @ xT[cc][:, qs]  + bq
            for name, wT, b_sb, dst in (
                ("q", wqT, bq_sb, qT),
                ("k", wkT, bk_sb, kT),
            ):
                for dc in range(ND):
                    for qs in range(NQ):
                        ps = psum_acc.tile([P, QS], F32, name=f"ps_{name}", tag="acc")
                        for cc in range(NC4):
                            nc.tensor.matmul(
                                ps,
                                wT[cc][:, dc * P : (dc + 1) * P],
                                xT[cc][:, qs * QS : (qs + 1) * QS],
                                start=(cc == 0),
                                stop=(cc == NC4 - 1),
                            )
                        nc.vector.tensor_scalar_add(
                            out=dst[dc][:, qs * QS : (qs + 1) * QS],
                            in0=ps,
                            scalar1=b_sb[:, dc : dc + 1],
                        )

            # v natural: v[tc] = sum_cc xT[cc][:,tc-block].T @ wvT[cc]  + bv
            for tch in range(NT):
                ps = psum_acc.tile([P, D], F32, name="ps_v", tag="acc")
                for cc in range(NC4):
                    nc.tensor.matmul(
                        ps,
                        xT[cc][:, tch * P : (tch + 1) * P],
                        wvT[cc],
                        start=(cc == 0),
                        stop=False,
                    )
                nc.tensor.matmul(ps, ones1, bv_sb, start=False, stop=True)
                nc.vector.tensor_copy(v[tch], ps)

        # ---- phase 2: scores (transposed) + column-softmax ----
        with tc.tile_pool(name="epool", bufs=1) as epool:
            e = [
                epool.tile([P, T - kc * P], mdt, name=f"e{kc}", tag=f"e{kc}")
                for kc in range(NT)
            ]

            for kc in range(NT):
                k0 = kc * P
                j0 = k0 // QS
                # q-slices covering the valid region [k0, T)
                slices = [(k0, (j0 + 1) * QS - k0)]
                for j in range(j0 + 1, NQ):
                    slices.append((j * QS, QS))
                ns = len(slices)

                maxs = stats.tile([P, NQ], F32, name="maxs", tag="maxs")
                sums = stats.tile([P, NQ], F32, name="sums", tag="sums")
                st_tiles = []
                for idx, (q0, w) in enumerate(slices):
                    st = psum_st.tile([P, w], F32, name="st", tag="st")
                    for dc in range(ND):
                        nc.tensor.matmul(
                            st,
                            kT[dc][:, k0 : k0 + P],
                            qT[dc][:, q0 : q0 + w],
                            start=(dc == 0),
                            stop=(dc == ND - 1),
                        )
                    if idx == 0:
                        # diagonal block: mask strict lower triangle (q < k)
                        nc.vector.tensor_add(st[:, 0:P], st[:, 0:P], tri)
                    nc.vector.reduce_max(
                        out=maxs[:, idx : idx + 1], in_=st,
                        axis=mybir.AxisListType.X,
                    )
                    st_tiles.append(st)

                m = stats.tile([P, 1], F32, name="m", tag="m")
                nc.vector.reduce_max(
                    out=m, in_=maxs[:, 0:ns], axis=mybir.AxisListType.X
                )
                nb = stats.tile([P, 1], F32, name="nb", tag="nb")
                nc.vector.tensor_scalar_mul(out=nb, in0=m, scalar1=-SCALE)

                for idx, (q0, w) in enumerate(slices):
                    nc.scalar.activation(
                        out=e[kc][:, q0 - k0 : q0 - k0 + w],
                        in_=st_tiles[idx],
                        func=mybir.ActivationFunctionType.Exp,
                        bias=nb,
                        scale=SCALE,
                        accum_out=sums[:, idx : idx + 1],
                    )

                S = stats.tile([P, 1], F32, name="S", tag="S")
                nc.vector.reduce_sum(
                    out=S, in_=sums[:, 0:ns], axis=mybir.AxisListType.X
                )
                rs = stats.tile([P, 1], F32, name="rs", tag="rs")
                nc.vector.reciprocal(out=rs, in_=S)
                # fold 1/S into v rows (normalizer is per-k == per-v-row)
                nc.scalar.mul(out=v[kc], in_=v[kc], mul=rs)

            # ---- phase 3: out[qc] = sum_{kc<=qc} e[kc][:, qc-block].T @ v[kc] ----
            for qc in range(NT):
                ps = psum_acc.tile([P, D], F32, name="ps_o", tag="acc")
                for kc in range(qc + 1):
                    off = (qc - kc) * P
                    nc.tensor.matmul(
                        ps,
                        e[kc][:, off : off + P],
                        v[kc],
                        start=(kc == 0),
                        stop=(kc == qc),
                    )
                osb = outsb.tile([P, D], F32, name="osb")
                nc.vector.tensor_copy(osb, ps)
                nc.sync.dma_start(
                    out=out[qc * P : (qc + 1) * P, C : 2 * C], in_=osb
                )


_NC_CACHE = {}


def _get_nc(mm_dtype="f32r"):
    if mm_dtype not in _NC_CACHE:
        _NC_CACHE[mm_dtype] = build_nc(mm_dtype)
    return _NC_CACHE[mm_dtype]


def kernel(**inputs):
    from concourse.bass_utils import run_bass_kernel_spmd

    nc = _get_nc()
    x = np.asarray(inputs["x"], dtype=np.float32)
    shared = {
        name: np.ascontiguousarray(np.asarray(inputs[name], dtype=np.float32))
        for name in ("Wq", "bq", "Wk", "bk", "Wv", "bv")
    }
    in_maps = [
        {"x": np.ascontiguousarray(x[b]), **shared} for b in range(B)
    ]
    res = run_bass_kernel_spmd(nc, in_maps, core_ids=list(range(B)))
    return np.stack([res.results[b]["out"] for b in range(B)], axis=0)
